# revision 13
# baseline (speedup 1.0000x reference)
"""YOLOv1 loss kernel for Trainium2, data-parallel over 8 NeuronCores.

Full inputs: pred [16384,30,7,7] f32, labels [16384,30,7,7] f32 -> scalar f32.

Sharding: batch 16384 -> 8 cores x 2048 rows. Host packs each core's shard
into bf16 channel-major images (geo: 19 channels, cls: 40 channels) so every
on-chip op is one wide contiguous span (DVE 2x bf16 mode). Per core the
kernel streams NCHUNK chunks, computes the loss fully on chip into two f32
accumulators per chunk ([P] each), and the host sums partials / divides by B.

Math (equivalent to the reference up to rounding):
  - Grid offsets m,n cancel in the IOU; scaling coords by 7 cancels in
    inter/union. Interval overlap identity: min(hi)-max(lo) =
    3.5*(wp+wg) - |xp-xg|, so no lo/hi box corners are ever formed.
  - inter = relu(ovx)*relu(ovy), den = 49*(a+ag) - inter, iou = inter/den
    (den >= 49*ag > 0 always).  1/den via exp(-ln(den)) on ACT.
  - (sqrt(p)-sqrt(l))^2 = p + l - 2*sqrt(p*l): e5 = 5*(p+l) - 10*sqrt(p*l),
    with 10*sqrt(m) = exp(0.5*ln(100*m)).
  - Per cell: cell = sph + obj*(selU + S + cls - sph), sph = 0.5(c1^2+c2^2),
    selU = resp ? cc1+0.5objc1 : cc2+0.5objc2, S = 0.5objc1+0.5objc2.
    Sum(sph) via ACT accumulate; Sum(obj*(...)) via tensor_tensor_reduce.
"""

import numpy as np
from ml_dtypes import bfloat16

import concourse.bass as bass
import concourse.mybir as mybir
import concourse.tile as tile
from concourse import bacc
from concourse.bass_utils import run_bass_kernel_spmd

F32 = mybir.dt.float32
BF16 = mybir.dt.bfloat16
I32 = mybir.dt.int32
OP = mybir.AluOpType
AF = mybir.ActivationFunctionType

NCORES = 8
B = 16384
BLOC = B // NCORES        # 2048 rows per core
P = 128                   # SBUF partitions
NCHUNK = 2
K = BLOC // P // NCHUNK   # 8 rows per partition per chunk
C = 49 * K                # 392 cells per partition per chunk
GCH = 19                  # geo channels
CCH = 40                  # cls channels (20 pred + 20 label)

SQ5 = float(np.float32(np.sqrt(5.0)))
ISQ2 = float(np.float32(np.sqrt(0.5)))

# how many cls channels the GPSIMD engine subtracts (rest on DVE)
GPS_SUB_CH = 4
# cls channels reduced by GPSIMD pool (window POOLW); rest tree-added on DVE
POOLW = 10


def _body(tc, geo_ap, cls_ap, out_ap):
    nc = tc.nc
    nv = nc.vector
    na = nc.scalar
    ng = nc.gpsimd

    import contextlib
    ctx = contextlib.ExitStack()
    with ctx:
        inp = ctx.enter_context(tc.tile_pool(name="inp", bufs=2))
        med = ctx.enter_context(tc.tile_pool(name="med", bufs=2))
        opool = ctx.enter_context(tc.tile_pool(name="opool", bufs=1))

        acc = opool.tile([P, 2 * NCHUNK], F32)

        for c in range(NCHUNK):
            GT = inp.tile([P, GCH * C], BF16, tag="GT")
            LT = inp.tile([P, CCH * C], BF16, tag="LT")
            nc.sync.dma_start(GT[:], geo_ap[c])
            nc.sync.dma_start(LT[:], cls_ap[c])

            def g(a, b):
                # geo channel span [a, b)
                return GT[:, a * C:b * C]

            def t2(name, ch, pool=med, dt=BF16):
                return pool.tile([P, ch * C], dt, tag=name, name=name)

            # ---- geometry: products / sums of widths (GPSIMD) ----
            M = t2("M", 4)       # [w1*lw, w2*lw, h1*lh, h2*lh]
            ng.tensor_tensor(M[:, 0:2 * C], g(4, 6), g(6, 8), OP.mult)
            ng.tensor_tensor(M[:, 2 * C:4 * C], g(8, 10), g(10, 12), OP.mult)
            AAG = t2("AAG", 4)   # [a1, a2, ag, ag]
            ng.tensor_tensor(AAG[:], g(4, 8), g(8, 12), OP.mult)

            D4 = t2("D4", 4)     # [x1-lx, x2-lx, y1-ly, y2-ly]
            nv.tensor_tensor(D4[:], g(0, 4), g(12, 16), OP.subtract)
            A4 = t2("A4", 4)     # |dxy| -> ov -> rv (in place)
            na.activation(A4[:], D4[:], AF.Abs)
            T4 = t2("T4", 4)     # [w1+lw, w2+lw, h1+lh, h2+lh]
            nv.tensor_tensor(T4[:, 0:2 * C], g(4, 6), g(6, 8), OP.add)
            nv.tensor_tensor(T4[:, 2 * C:4 * C], g(8, 10), g(10, 12), OP.add)

            # s'' = 10*sqrt(m) = exp(0.5*ln(100*m))   (in place over M)
            na.activation(M[:], M[:], AF.Ln, scale=100.0)
            na.activation(M[:], M[:], AF.Exp, scale=0.5)

            # ---- intersection / iou ----
            # ov = min(3.5*t - |dxy|, 7*min(wp,wg)); rv = relu(ov)
            MN = t2("MN", 4)
            nv.tensor_tensor(MN[:, 0:2 * C], g(4, 6), g(6, 8), OP.min)
            nv.tensor_tensor(MN[:, 2 * C:4 * C], g(8, 10), g(10, 12), OP.min)
            nv.scalar_tensor_tensor(A4[:], T4[:], 3.5, A4[:], OP.mult,
                                    OP.subtract)
            nv.scalar_tensor_tensor(A4[:], MN[:], 7.0, A4[:], OP.mult,
                                    OP.min)
            nv.tensor_scalar_max(A4[:], A4[:], 0.0)
            I2 = t2("I2", 2)     # inter -> iou (in place)
            nv.tensor_tensor(I2[:], A4[:, 0:2 * C], A4[:, 2 * C:4 * C],
                             OP.mult)
            S2 = t2("S2", 2)     # a+ag -> den -> ln -> 1/den (in place)
            nv.tensor_tensor(S2[:], AAG[:, 0:2 * C], AAG[:, 2 * C:4 * C],
                             OP.add)
            nv.scalar_tensor_tensor(S2[:], S2[:], 49.0, I2[:], OP.mult,
                                    OP.subtract)
            na.activation(S2[:], S2[:], AF.Ln)
            na.activation(S2[:], S2[:], AF.Exp, scale=-1.0)
            nv.tensor_tensor(I2[:], I2[:], S2[:], OP.mult)   # iou1, iou2

            RSP = t2("RSP", 1, dt=I32)
            nv.tensor_tensor(RSP[:], I2[:, 0:C], I2[:, C:2 * C], OP.is_ge)

            # objc' = 0.5*(c - iou)^2
            DC = t2("DC", 2)
            nv.tensor_tensor(DC[:], g(16, 18), I2[:], OP.subtract)
            na.activation(DC[:], DC[:], AF.Square, scale=ISQ2)

            # ---- coordinate loss ----
            DQ = t2("DQ", 8)     # [5*dxy^2 (4ch) | e5 (4ch)]
            na.activation(DQ[:, 0:4 * C], D4[:], AF.Square, scale=SQ5)
            nv.scalar_tensor_tensor(DQ[:, 4 * C:8 * C], T4[:], 5.0, M[:],
                                    OP.mult, OP.subtract)
            nv.tensor_tensor(DQ[:, 0:4 * C], DQ[:, 0:4 * C],
                             DQ[:, 4 * C:8 * C], OP.add)
            nv.tensor_tensor(DQ[:, 0:2 * C], DQ[:, 0:2 * C],
                             DQ[:, 2 * C:4 * C], OP.add)    # cc1, cc2
            nv.tensor_tensor(DQ[:, 0:2 * C], DQ[:, 0:2 * C], DC[:],
                             OP.add)                        # cc_k + objc'_k

            SS1 = t2("SS1", 1)   # S = objc'1 + objc'2
            ng.tensor_tensor(SS1[:], DC[:, 0:C], DC[:, C:2 * C], OP.add)

            # sph = 0.5*(c1^2 + c2^2); total into acc col, per-cell via GPS
            H2 = t2("H2", 2)
            na.activation(H2[:], g(16, 18), AF.Square, scale=ISQ2)
            nv.tensor_reduce(acc[:, 2 * c:2 * c + 1], H2[:],
                             mybir.AxisListType.X, OP.add)
            SPH = t2("SPH", 1)
            ng.tensor_tensor(SPH[:], H2[:, 0:C], H2[:, C:2 * C], OP.add)

            # ---- cls ----
            D20 = t2("D20", 20)
            nde = 20 - GPS_SUB_CH
            nv.tensor_tensor(D20[:, 0:nde * C], LT[:, 0:nde * C],
                             LT[:, 20 * C:(20 + nde) * C], OP.subtract)
            ng.tensor_tensor(D20[:, nde * C:20 * C],
                             LT[:, nde * C:20 * C],
                             LT[:, (20 + nde) * C:40 * C], OP.subtract)
            na.activation(D20[:], D20[:], AF.Square)

            # channel reduce tree: GPS takes one 5ch level, DVE the rest
            o = 10 * C
            ng.tensor_tensor(D20[:, 0:5 * C], D20[:, 0:5 * C],
                             D20[:, 5 * C:10 * C], OP.add)   # s10a (GPS)
            nv.tensor_tensor(D20[:, o:o + 5 * C], D20[:, o:o + 5 * C],
                             D20[:, o + 5 * C:o + 10 * C], OP.add)  # s10b
            nv.tensor_tensor(D20[:, 0:5 * C], D20[:, 0:5 * C],
                             D20[:, o:o + 5 * C], OP.add)    # s5
            nv.tensor_tensor(D20[:, 0:2 * C], D20[:, 0:2 * C],
                             D20[:, 2 * C:4 * C], OP.add)
            nv.tensor_tensor(D20[:, 0:C], D20[:, 0:C],
                             D20[:, C:2 * C], OP.add)
            nv.tensor_tensor(D20[:, 0:C], D20[:, 0:C],
                             D20[:, 4 * C:5 * C], OP.add)    # cls total

            # ---- combine ----
            V1 = t2("V1", 1)
            nv.tensor_copy(V1[:], DQ[:, C:2 * C])
            nv.copy_predicated(V1[:], RSP[:], DQ[:, 0:C])
            nv.tensor_tensor(V1[:], V1[:], SS1[:], OP.add)
            nv.tensor_tensor(V1[:], V1[:], D20[:, 0:C], OP.add)
            nv.tensor_tensor(V1[:], V1[:], SPH[:], OP.subtract)
            W4 = t2("W4", 1)
            nv.tensor_tensor(W4[:], V1[:], g(18, 19), OP.mult)
            nv.tensor_reduce(acc[:, 2 * c + 1:2 * c + 2], W4[:],
                             mybir.AxisListType.X, OP.add)

        nc.sync.dma_start(out_ap, acc[:])


_NC_CACHE = None


def build_nc():
    global _NC_CACHE
    if _NC_CACHE is not None:
        return _NC_CACHE
    nc = bacc.Bacc(
        "TRN2",
        target_bir_lowering=False,
        debug=False,
        enable_asserts=False,
        num_devices=NCORES,
    )
    geo = nc.dram_tensor("geo", [NCHUNK, P, GCH * C], BF16,
                         kind="ExternalInput")
    cls = nc.dram_tensor("cls", [NCHUNK, P, CCH * C], BF16,
                         kind="ExternalInput")
    out = nc.dram_tensor("out", [P, 2 * NCHUNK], F32, kind="ExternalOutput")
    with tile.TileContext(nc) as tc:
        _body(tc, geo.ap(), cls.ap(), out.ap())
    nc.compile()
    _NC_CACHE = nc
    return nc


def make_in_maps(pred, labels):
    pred = np.asarray(pred, dtype=np.float32).reshape(B, 30, 49)
    labels = np.asarray(labels, dtype=np.float32).reshape(B, 30, 49)
    # [B,ch,49] -> [core, chunk, k, p, ch, 49] -> [core, chunk, p, ch, k, 49]
    def img(x, chans):
        x = x[:, chans]                       # [B, nch, 49]
        n = len(chans)
        x = x.reshape(NCORES, NCHUNK, K, P, n, 49)
        x = x.transpose(0, 1, 3, 4, 2, 5)     # core, chunk, p, ch, k, 49
        return np.ascontiguousarray(x).reshape(NCORES, NCHUNK, P, n * C)

    pg = [0, 5, 1, 6,          # x1 x2 y1 y2
          2, 7,                # w1 w2
          3, 8,                # h1 h2
          4, 9]                # c1 c2
    geo = np.empty((NCORES, NCHUNK, P, GCH, C), dtype=np.float32)
    gp = img(pred, pg).reshape(NCORES, NCHUNK, P, 10, C)
    gl = img(labels, [0, 1, 2, 3, 4]).reshape(NCORES, NCHUNK, P, 5, C)
    geo[:, :, :, 0:4] = gp[:, :, :, 0:4]                   # x1 x2 y1 y2
    geo[:, :, :, 4:6] = gp[:, :, :, 4:6]                   # w1 w2
    geo[:, :, :, 6] = gl[:, :, :, 2]                       # lw
    geo[:, :, :, 7] = gl[:, :, :, 2]                       # lw
    geo[:, :, :, 8:10] = gp[:, :, :, 6:8]                  # h1 h2
    geo[:, :, :, 10] = gl[:, :, :, 3]                      # lh
    geo[:, :, :, 11] = gl[:, :, :, 3]                      # lh
    geo[:, :, :, 12] = gl[:, :, :, 0]                      # lx
    geo[:, :, :, 13] = gl[:, :, :, 0]                      # lx
    geo[:, :, :, 14] = gl[:, :, :, 1]                      # ly
    geo[:, :, :, 15] = gl[:, :, :, 1]                      # ly
    geo[:, :, :, 16:18] = gp[:, :, :, 8:10]                # c1 c2
    geo[:, :, :, 18] = gl[:, :, :, 4]                      # obj
    geo = geo.reshape(NCORES, NCHUNK, P, GCH * C).astype(bfloat16)
    clsi = img(pred, list(range(10, 30)))
    clsl = img(labels, list(range(10, 30)))
    cls = np.concatenate(
        [clsi.reshape(NCORES, NCHUNK, P, 20, C),
         clsl.reshape(NCORES, NCHUNK, P, 20, C)], axis=3
    ).reshape(NCORES, NCHUNK, P, CCH * C).astype(bfloat16)
    return [
        {"geo": np.ascontiguousarray(geo[i]),
         "cls": np.ascontiguousarray(cls[i])}
        for i in range(NCORES)
    ]


def run(pred, labels, trace=False, **kw):
    nc = build_nc()
    in_maps = make_in_maps(pred, labels)
    res = run_bass_kernel_spmd(
        nc, in_maps, core_ids=list(range(NCORES)), trace=trace, **kw)
    total = np.float64(0.0)
    for r in res.results:
        total += r["out"].astype(np.float64).sum()
    loss = np.float32(total / B)
    return loss, res


def kernel(pred, labels):
    loss, _ = run(pred, labels)
    return np.array(loss, dtype=np.float32)


# revision 16
# speedup vs baseline: 1.3997x; 1.3997x over previous
"""YOLOv1 loss kernel for Trainium2, data-parallel over 8 NeuronCores.

Full inputs: pred [16384,30,7,7] f32, labels [16384,30,7,7] f32 -> scalar f32.

Sharding: batch 16384 -> 8 cores x 2048 rows. Host packs each core's shard
into bf16 channel-major images (geo: 19 channels, cls: 40 channels) so every
on-chip op is one wide contiguous span (DVE 2x bf16 mode). Per core the
kernel streams NCHUNK chunks, computes the loss fully on chip into two f32
accumulators per chunk ([P] each), and the host sums partials / divides by B.

Math (equivalent to the reference up to rounding):
  - Grid offsets m,n cancel in the IOU; scaling coords by 7 cancels in
    inter/union. Interval overlap identity: min(hi)-max(lo) =
    3.5*(wp+wg) - |xp-xg|, so no lo/hi box corners are ever formed.
  - inter = relu(ovx)*relu(ovy), den = 49*(a+ag) - inter, iou = inter/den
    (den >= 49*ag > 0 always).  1/den via exp(-ln(den)) on ACT.
  - (sqrt(p)-sqrt(l))^2 = p + l - 2*sqrt(p*l): e5 = 5*(p+l) - 10*sqrt(p*l),
    with 10*sqrt(m) = exp(0.5*ln(100*m)).
  - Per cell: cell = sph + obj*(selU + S + cls - sph), sph = 0.5(c1^2+c2^2),
    selU = resp ? cc1+0.5objc1 : cc2+0.5objc2, S = 0.5objc1+0.5objc2.
    Sum(sph) via ACT accumulate; Sum(obj*(...)) via tensor_tensor_reduce.
"""

import numpy as np
from ml_dtypes import bfloat16

import concourse.bass as bass
import concourse.mybir as mybir
import concourse.tile as tile
from concourse import bacc
from concourse.bass_utils import run_bass_kernel_spmd

F32 = mybir.dt.float32
BF16 = mybir.dt.bfloat16
I32 = mybir.dt.int32
OP = mybir.AluOpType
AF = mybir.ActivationFunctionType

NCORES = 8
B = 16384
BLOC = B // NCORES        # 2048 rows per core
P = 128                   # SBUF partitions
NCHUNK = 2
K = BLOC // P // NCHUNK   # 8 rows per partition per chunk
C = 49 * K                # 392 cells per partition per chunk
GCH = 19                  # geo channels
CCH = 40                  # cls channels (20 pred + 20 label)

SQ5 = float(np.float32(np.sqrt(5.0)))
ISQ2 = float(np.float32(np.sqrt(0.5)))

# how many cls channels the GPSIMD engine subtracts (rest on DVE)
GPS_SUB_CH = 4
# cls channels reduced by GPSIMD pool (window POOLW); rest tree-added on DVE
POOLW = 10


def _body(tc, geo_ap, cls_ap, out_ap):
    nc = tc.nc
    nv = nc.vector
    na = nc.scalar
    ng = nc.gpsimd

    import contextlib
    ctx = contextlib.ExitStack()
    with ctx:
        inp = ctx.enter_context(tc.tile_pool(name="inp", bufs=2))
        med = ctx.enter_context(tc.tile_pool(name="med", bufs=2))
        opool = ctx.enter_context(tc.tile_pool(name="opool", bufs=1))

        acc = opool.tile([P, 2 * NCHUNK], F32)

        for c in range(NCHUNK):
            GT = inp.tile([P, GCH * C], BF16, tag="GT")
            LT = inp.tile([P, CCH * C], BF16, tag="LT")
            nc.sync.dma_start(GT[:], geo_ap[c])
            nc.sync.dma_start(LT[:], cls_ap[c])

            def g(a, b):
                # geo channel span [a, b)
                return GT[:, a * C:b * C]

            def t2(name, ch, pool=med, dt=BF16):
                return pool.tile([P, ch * C], dt, tag=name, name=name)

            # ---- geometry ----
            D4 = t2("D4", 4)     # [x1-lx, x2-lx, y1-ly, y2-ly]
            nv.tensor_tensor(D4[:], g(0, 4), g(12, 16), OP.subtract)
            A4 = t2("A4", 4)     # |dxy|/3.5 -> ov -> rv (in place)
            na.activation(A4[:], D4[:], AF.Abs, scale=1.0 / 3.5)
            T4 = t2("T4", 4)     # [w1+lw, w2+lw, h1+lh, h2+lh]
            nv.tensor_tensor(T4[:, 0:2 * C], g(4, 6), g(6, 8), OP.add)
            nv.tensor_tensor(T4[:, 2 * C:4 * C], g(8, 10), g(10, 12), OP.add)
            M = t2("M", 4)       # [w1*lw, w2*lw, h1*lh, h2*lh]
            nv.tensor_tensor(M[:, 0:2 * C], g(4, 6), g(6, 8), OP.mult)
            nv.tensor_tensor(M[:, 2 * C:4 * C], g(8, 10), g(10, 12), OP.mult)
            AAG = t2("AAG", 4)   # [a1, a2, ag, ag]
            nv.tensor_tensor(AAG[:], g(4, 8), g(8, 12), OP.mult)

            # 2*sqrt(m) = exp(0.5*ln(4*m))   (in place over M)
            na.activation(M[:], M[:], AF.Ln, scale=4.0)
            na.activation(M[:], M[:], AF.Exp, scale=0.5)

            # ---- intersection / iou (coords scaled by 1/3.5) ----
            # ovs = min(t - |dxy|/3.5, 2*min(wp,wg)); rv = relu(ovs)
            MN = t2("MN", 4)
            nv.tensor_tensor(MN[:, 0:2 * C], g(4, 6), g(6, 8), OP.min)
            nv.tensor_tensor(MN[:, 2 * C:4 * C], g(8, 10), g(10, 12), OP.min)
            nv.tensor_tensor(A4[:], T4[:], A4[:], OP.subtract)
            nv.scalar_tensor_tensor(A4[:], MN[:], 2.0, A4[:], OP.mult,
                                    OP.min)
            nv.tensor_scalar_max(A4[:], A4[:], 0.0)
            I2 = t2("I2", 2)     # inter/12.25 -> iou (in place)
            nv.tensor_tensor(I2[:], A4[:, 0:2 * C], A4[:, 2 * C:4 * C],
                             OP.mult)
            S2 = t2("S2", 2)     # a+ag -> den/12.25 -> ln -> 1/den (in place)
            nv.tensor_tensor(S2[:], AAG[:, 0:2 * C], AAG[:, 2 * C:4 * C],
                             OP.add)
            nv.scalar_tensor_tensor(S2[:], S2[:], 4.0, I2[:], OP.mult,
                                    OP.subtract)
            na.activation(S2[:], S2[:], AF.Ln)
            na.activation(S2[:], S2[:], AF.Exp, scale=-1.0)
            nv.tensor_tensor(I2[:], I2[:], S2[:], OP.mult)   # iou1, iou2

            RSP = t2("RSP", 1, dt=I32)
            nv.tensor_tensor(RSP[:], I2[:, 0:C], I2[:, C:2 * C], OP.is_ge)

            # objc'' = 0.1*(c - iou)^2
            DC = t2("DC", 2)
            nv.tensor_tensor(DC[:], g(16, 18), I2[:], OP.subtract)
            na.activation(DC[:], DC[:], AF.Square,
                          scale=float(np.sqrt(0.1)))

            # ---- coordinate loss (unscaled; the x5 is folded in at w2) ----
            DQ = t2("DQ", 8)     # [dxy^2 (4ch) | t - 2*sqrt(m) (4ch)]
            na.activation(DQ[:, 0:4 * C], D4[:], AF.Square)
            nv.tensor_tensor(DQ[:, 4 * C:8 * C], T4[:], M[:], OP.subtract)
            nv.tensor_tensor(DQ[:, 0:4 * C], DQ[:, 0:4 * C],
                             DQ[:, 4 * C:8 * C], OP.add)
            nv.tensor_tensor(DQ[:, 0:2 * C], DQ[:, 0:2 * C],
                             DQ[:, 2 * C:4 * C], OP.add)    # coor1, coor2
            nv.tensor_tensor(DQ[:, 0:2 * C], DQ[:, 0:2 * C], DC[:],
                             OP.add)                # coor_k + objc''_k

            SS1 = t2("SS1", 1)   # S'' = objc''1 + objc''2
            nv.tensor_tensor(SS1[:], DC[:, 0:C], DC[:, C:2 * C], OP.add)

            # sph = 0.5*(c1^2 + c2^2): total into acc col + per-cell
            H2 = t2("H2", 2)
            na.activation(H2[:], g(16, 18), AF.Square, scale=ISQ2)
            nv.tensor_reduce(acc[:, 2 * c:2 * c + 1], H2[:],
                             mybir.AxisListType.X, OP.add)
            SPH = t2("SPH", 1)
            nv.tensor_tensor(SPH[:], H2[:, 0:C], H2[:, C:2 * C], OP.add)

            # ---- cls: two independent 10-channel pipelines ----
            D20 = t2("D20", 20)
            for h in range(2):
                o = 10 * h * C
                lo, lp = o, 20 * C + o
                nv.tensor_tensor(D20[:, o:o + 10 * C], LT[:, lo:lo + 10 * C],
                                 LT[:, lp:lp + 10 * C], OP.subtract)
                na.activation(D20[:, o:o + 10 * C], D20[:, o:o + 10 * C],
                              AF.Square)
                nv.tensor_tensor(D20[:, o:o + 5 * C], D20[:, o:o + 5 * C],
                                 D20[:, o + 5 * C:o + 10 * C], OP.add)
                nv.tensor_tensor(D20[:, o:o + 2 * C], D20[:, o:o + 2 * C],
                                 D20[:, o + 2 * C:o + 4 * C], OP.add)
                nv.tensor_tensor(D20[:, o:o + C], D20[:, o:o + C],
                                 D20[:, o + C:o + 2 * C], OP.add)
                nv.tensor_tensor(D20[:, o:o + C], D20[:, o:o + C],
                                 D20[:, o + 4 * C:o + 5 * C], OP.add)
            nv.tensor_tensor(D20[:, 0:C], D20[:, 0:C],
                             D20[:, 10 * C:11 * C], OP.add)  # cls total

            # ---- combine:
            # w = 5*(sel(resp, cor1+.1objc1, cor2+.1objc2) + S'') + cls - sph
            V1 = t2("V1", 1)
            nv.tensor_copy(V1[:], DQ[:, C:2 * C])
            nv.copy_predicated(V1[:], RSP[:], DQ[:, 0:C])
            nv.tensor_tensor(V1[:], V1[:], SS1[:], OP.add)
            nv.scalar_tensor_tensor(V1[:], V1[:], 5.0, D20[:, 0:C],
                                    OP.mult, OP.add)
            nv.tensor_tensor(V1[:], V1[:], SPH[:], OP.subtract)
            W4 = t2("W4", 1)
            nv.tensor_tensor(W4[:], V1[:], g(18, 19), OP.mult)
            nv.tensor_reduce(acc[:, 2 * c + 1:2 * c + 2], W4[:],
                             mybir.AxisListType.X, OP.add)

        nc.sync.dma_start(out_ap, acc[:])


_NC_CACHE = None


def build_nc():
    global _NC_CACHE
    if _NC_CACHE is not None:
        return _NC_CACHE
    nc = bacc.Bacc(
        "TRN2",
        target_bir_lowering=False,
        debug=False,
        enable_asserts=False,
        num_devices=NCORES,
    )
    geo = nc.dram_tensor("geo", [NCHUNK, P, GCH * C], BF16,
                         kind="ExternalInput")
    cls = nc.dram_tensor("cls", [NCHUNK, P, CCH * C], BF16,
                         kind="ExternalInput")
    out = nc.dram_tensor("out", [P, 2 * NCHUNK], F32, kind="ExternalOutput")
    with tile.TileContext(nc) as tc:
        _body(tc, geo.ap(), cls.ap(), out.ap())
    nc.compile()
    _NC_CACHE = nc
    return nc


def make_in_maps(pred, labels):
    pred = np.asarray(pred, dtype=np.float32).reshape(B, 30, 49)
    labels = np.asarray(labels, dtype=np.float32).reshape(B, 30, 49)
    # [B,ch,49] -> [core, chunk, k, p, ch, 49] -> [core, chunk, p, ch, k, 49]
    def img(x, chans):
        x = x[:, chans]                       # [B, nch, 49]
        n = len(chans)
        x = x.reshape(NCORES, NCHUNK, K, P, n, 49)
        x = x.transpose(0, 1, 3, 4, 2, 5)     # core, chunk, p, ch, k, 49
        return np.ascontiguousarray(x).reshape(NCORES, NCHUNK, P, n * C)

    pg = [0, 5, 1, 6,          # x1 x2 y1 y2
          2, 7,                # w1 w2
          3, 8,                # h1 h2
          4, 9]                # c1 c2
    geo = np.empty((NCORES, NCHUNK, P, GCH, C), dtype=np.float32)
    gp = img(pred, pg).reshape(NCORES, NCHUNK, P, 10, C)
    gl = img(labels, [0, 1, 2, 3, 4]).reshape(NCORES, NCHUNK, P, 5, C)
    geo[:, :, :, 0:4] = gp[:, :, :, 0:4]                   # x1 x2 y1 y2
    geo[:, :, :, 4:6] = gp[:, :, :, 4:6]                   # w1 w2
    geo[:, :, :, 6] = gl[:, :, :, 2]                       # lw
    geo[:, :, :, 7] = gl[:, :, :, 2]                       # lw
    geo[:, :, :, 8:10] = gp[:, :, :, 6:8]                  # h1 h2
    geo[:, :, :, 10] = gl[:, :, :, 3]                      # lh
    geo[:, :, :, 11] = gl[:, :, :, 3]                      # lh
    geo[:, :, :, 12] = gl[:, :, :, 0]                      # lx
    geo[:, :, :, 13] = gl[:, :, :, 0]                      # lx
    geo[:, :, :, 14] = gl[:, :, :, 1]                      # ly
    geo[:, :, :, 15] = gl[:, :, :, 1]                      # ly
    geo[:, :, :, 16:18] = gp[:, :, :, 8:10]                # c1 c2
    geo[:, :, :, 18] = gl[:, :, :, 4]                      # obj
    geo = geo.reshape(NCORES, NCHUNK, P, GCH * C).astype(bfloat16)
    clsi = img(pred, list(range(10, 30)))
    clsl = img(labels, list(range(10, 30)))
    cls = np.concatenate(
        [clsi.reshape(NCORES, NCHUNK, P, 20, C),
         clsl.reshape(NCORES, NCHUNK, P, 20, C)], axis=3
    ).reshape(NCORES, NCHUNK, P, CCH * C).astype(bfloat16)
    return [
        {"geo": np.ascontiguousarray(geo[i]),
         "cls": np.ascontiguousarray(cls[i])}
        for i in range(NCORES)
    ]


def run(pred, labels, trace=False, **kw):
    nc = build_nc()
    in_maps = make_in_maps(pred, labels)
    res = run_bass_kernel_spmd(
        nc, in_maps, core_ids=list(range(NCORES)), trace=trace, **kw)
    total = np.float64(0.0)
    for r in res.results:
        total += r["out"].astype(np.float64).sum()
    loss = np.float32(total / B)
    return loss, res


def kernel(pred, labels):
    loss, _ = run(pred, labels)
    return np.array(loss, dtype=np.float32)


# revision 24
# speedup vs baseline: 1.4089x; 1.0065x over previous
"""YOLOv1 loss kernel for Trainium2, data-parallel over 8 NeuronCores.

Full inputs: pred [16384,30,7,7] f32, labels [16384,30,7,7] f32 -> scalar f32.

Sharding: batch 16384 -> 8 cores x 2048 rows. Host packs each core's shard
into bf16 channel-major images (geo: 19 channels, cls: 40 channels) so every
on-chip op is one wide contiguous span (DVE 2x bf16 mode). Per core the
kernel streams NCHUNK chunks, computes the loss fully on chip into two f32
accumulators per chunk ([P] each), and the host sums partials / divides by B.

Math (equivalent to the reference up to rounding):
  - Grid offsets m,n cancel in the IOU; scaling coords by 7 cancels in
    inter/union. Interval overlap identity: min(hi)-max(lo) =
    3.5*(wp+wg) - |xp-xg|, so no lo/hi box corners are ever formed.
  - inter = relu(ovx)*relu(ovy), den = 49*(a+ag) - inter, iou = inter/den
    (den >= 49*ag > 0 always).  1/den via exp(-ln(den)) on ACT.
  - (sqrt(p)-sqrt(l))^2 = p + l - 2*sqrt(p*l): e5 = 5*(p+l) - 10*sqrt(p*l),
    with 10*sqrt(m) = exp(0.5*ln(100*m)).
  - Per cell: cell = sph + obj*(selU + S + cls - sph), sph = 0.5(c1^2+c2^2),
    selU = resp ? cc1+0.5objc1 : cc2+0.5objc2, S = 0.5objc1+0.5objc2.
    Sum(sph) via ACT accumulate; Sum(obj*(...)) via tensor_tensor_reduce.
"""

import numpy as np
from ml_dtypes import bfloat16

import concourse.bass as bass
import concourse.mybir as mybir
import concourse.tile as tile
from concourse import bacc
from concourse.bass_utils import run_bass_kernel_spmd

F32 = mybir.dt.float32
BF16 = mybir.dt.bfloat16
I32 = mybir.dt.int32
OP = mybir.AluOpType
AF = mybir.ActivationFunctionType

NCORES = 8
B = 16384
BLOC = B // NCORES        # 2048 rows per core
P = 128                   # SBUF partitions
KS = [5, 11]              # rows per partition per chunk (sum = BLOC/P)
NCHUNK = len(KS)
CS = [49 * k for k in KS]  # cells per partition per chunk
GCH = 19                  # geo channels
CCH = 40                  # cls channels (20 pred + 20 label)

SQ5 = float(np.float32(np.sqrt(5.0)))
ISQ2 = float(np.float32(np.sqrt(0.5)))

# how many cls channels the GPSIMD engine subtracts (rest on DVE)
GPS_SUB_CH = 4
# cls channels reduced by GPSIMD pool (window POOLW); rest tree-added on DVE
POOLW = 10


def _body(tc, geo_ap, cls_ap, out_ap):
    nc = tc.nc
    nv = nc.vector
    na = nc.scalar
    ng = nc.gpsimd

    import contextlib
    ctx = contextlib.ExitStack()
    with ctx:
        inp = ctx.enter_context(tc.tile_pool(name="inp", bufs=2))
        med = ctx.enter_context(tc.tile_pool(name="med", bufs=1))
        opool = ctx.enter_context(tc.tile_pool(name="opool", bufs=1))

        acc = opool.tile([P, 2 * NCHUNK], F32)
        CMAX = max(CS)

        for c in range(NCHUNK):
            C = CS[c]
            go = GCH * sum(CS[:c])
            lo_off = CCH * sum(CS[:c])
            GT = inp.tile([P, GCH * CMAX], BF16, tag="GT")
            LT = inp.tile([P, CCH * CMAX], BF16, tag="LT")
            nc.sync.dma_start(GT[:, 0:GCH * C], geo_ap[:, go:go + GCH * C])
            nc.sync.dma_start(LT[:, 0:CCH * C],
                              cls_ap[:, lo_off:lo_off + CCH * C])

            def g(a, b):
                # geo channel span [a, b)
                return GT[:, a * C:b * C]

            def t2(name, ch, pool=med, dt=BF16):
                t = pool.tile([P, ch * CMAX], dt, tag=name, name=name)
                return t[:, 0:ch * C]

            # ---- geometry ----
            D4 = t2("D4", 4)     # [x1-lx, x2-lx, y1-ly, y2-ly]
            nv.tensor_tensor(D4[:], g(0, 4), g(12, 16), OP.subtract)
            A4 = t2("A4", 4)     # |dxy|/3.5 -> ov -> rv (in place)
            na.activation(A4[:], D4[:], AF.Abs, scale=1.0 / 3.5)
            # w/h channel pair views: [[w1 w2],[h1 h2]] and [[lw lw],[lh lh]]
            gv = GT[:, 4 * C:12 * C].rearrange("p (b x) -> p b x", b=2)
            wh_p = gv[:, :, 0:2 * C]
            wh_l = gv[:, :, 2 * C:4 * C]

            def bv(t):
                return t.rearrange("p (b x) -> p b x", b=2)

            T4 = t2("T4", 4)     # [w1+lw, w2+lw, h1+lh, h2+lh]
            nv.tensor_tensor(bv(T4), wh_p, wh_l, OP.add)
            M = t2("M", 4)       # [w1*lw, w2*lw, h1*lh, h2*lh]
            nv.tensor_tensor(bv(M), wh_p, wh_l, OP.mult)
            AAG = t2("AAG", 4)   # [a1, a2, ag, ag]
            nv.tensor_tensor(AAG[:], g(4, 8), g(8, 12), OP.mult)

            # 2*sqrt(m) = exp(0.5*ln(4*m))   (in place over M)
            na.activation(M[:], M[:], AF.Ln, scale=4.0)
            na.activation(M[:], M[:], AF.Exp, scale=0.5)

            # ---- intersection / iou (coords scaled by 1/3.5) ----
            # ovs = min(t - |dxy|/3.5, 2*min(wp,wg)); rv = relu(ovs)
            MN = t2("MN", 4)
            nv.tensor_tensor(bv(MN), wh_p, wh_l, OP.min)
            nv.tensor_tensor(A4[:], T4[:], A4[:], OP.subtract)
            nv.scalar_tensor_tensor(A4[:], MN[:], 2.0, A4[:], OP.mult,
                                    OP.min)
            nv.tensor_scalar_max(A4[:], A4[:], 0.0)
            I2 = t2("I2", 2)     # inter/12.25 -> iou (in place)
            nv.tensor_tensor(I2[:], A4[:, 0:2 * C], A4[:, 2 * C:4 * C],
                             OP.mult)
            S2 = t2("S2", 2)     # a+ag -> den/12.25 -> ln -> 1/den (in place)
            nv.tensor_tensor(S2[:], AAG[:, 0:2 * C], AAG[:, 2 * C:4 * C],
                             OP.add)
            nv.scalar_tensor_tensor(S2[:], S2[:], 4.0, I2[:], OP.mult,
                                    OP.subtract)
            na.activation(S2[:], S2[:], AF.Ln)
            na.activation(S2[:], S2[:], AF.Exp, scale=-1.0)
            nv.tensor_tensor(I2[:], I2[:], S2[:], OP.mult)   # iou1, iou2

            RSP = t2("RSP", 1, dt=I32)
            nv.tensor_tensor(RSP[:], I2[:, 0:C], I2[:, C:2 * C], OP.is_ge)

            # objc'' = 0.1*(c - iou)^2
            DC = t2("DC", 2)
            nv.tensor_tensor(DC[:], g(16, 18), I2[:], OP.subtract)
            na.activation(DC[:], DC[:], AF.Square,
                          scale=float(np.sqrt(0.1)))

            # ---- coordinate loss (unscaled; the x5 is folded in at w2) ----
            DQ = t2("DQ", 8)     # [dxy^2 (4ch) | t - 2*sqrt(m) (4ch)]
            na.activation(DQ[:, 0:4 * C], D4[:], AF.Square)
            nv.tensor_tensor(DQ[:, 4 * C:8 * C], T4[:], M[:], OP.subtract)
            nv.tensor_tensor(DQ[:, 0:4 * C], DQ[:, 0:4 * C],
                             DQ[:, 4 * C:8 * C], OP.add)
            nv.tensor_tensor(DQ[:, 0:2 * C], DQ[:, 0:2 * C],
                             DQ[:, 2 * C:4 * C], OP.add)    # coor1, coor2
            nv.tensor_tensor(DQ[:, 0:2 * C], DQ[:, 0:2 * C], DC[:],
                             OP.add)                # coor_k + objc''_k

            SS1 = t2("SS1", 1)   # S'' = objc''1 + objc''2
            nv.tensor_tensor(SS1[:], DC[:, 0:C], DC[:, C:2 * C], OP.add)

            # sph = 0.5*(c1^2 + c2^2): total into acc col + per-cell
            H2 = t2("H2", 2)
            na.activation(H2[:], g(16, 18), AF.Square, scale=ISQ2)
            nv.tensor_reduce(acc[:, 2 * c:2 * c + 1], H2[:],
                             mybir.AxisListType.X, OP.add)
            SPH = t2("SPH", 1)
            nv.tensor_tensor(SPH[:], H2[:, 0:C], H2[:, C:2 * C], OP.add)

            # ---- cls: two independent 10-channel pipelines ----
            D20 = t2("D20", 20)
            for h in range(2):
                o = 10 * h * C
                lo, lp = o, 20 * C + o
                nv.tensor_tensor(D20[:, o:o + 10 * C], LT[:, lo:lo + 10 * C],
                                 LT[:, lp:lp + 10 * C], OP.subtract)
                na.activation(D20[:, o:o + 10 * C], D20[:, o:o + 10 * C],
                              AF.Square)
                nv.tensor_tensor(D20[:, o:o + 5 * C], D20[:, o:o + 5 * C],
                                 D20[:, o + 5 * C:o + 10 * C], OP.add)
                nv.tensor_tensor(D20[:, o:o + 2 * C], D20[:, o:o + 2 * C],
                                 D20[:, o + 2 * C:o + 4 * C], OP.add)
                nv.tensor_tensor(D20[:, o:o + C], D20[:, o:o + C],
                                 D20[:, o + C:o + 2 * C], OP.add)
                nv.tensor_tensor(D20[:, o:o + C], D20[:, o:o + C],
                                 D20[:, o + 4 * C:o + 5 * C], OP.add)
            nv.tensor_tensor(D20[:, 0:C], D20[:, 0:C],
                             D20[:, 10 * C:11 * C], OP.add)  # cls total

            # ---- combine:
            # w = 5*(sel(resp, cor1+.1objc1, cor2+.1objc2) + S'') + cls - sph
            V1 = t2("V1", 1)
            nv.tensor_copy(V1[:], DQ[:, C:2 * C])
            nv.copy_predicated(V1[:], RSP[:], DQ[:, 0:C])
            nv.tensor_tensor(V1[:], V1[:], SS1[:], OP.add)
            nv.scalar_tensor_tensor(V1[:], V1[:], 5.0, D20[:, 0:C],
                                    OP.mult, OP.add)
            nv.tensor_tensor(V1[:], V1[:], SPH[:], OP.subtract)
            W4 = t2("W4", 1)
            nv.tensor_tensor(W4[:], V1[:], g(18, 19), OP.mult)
            nv.tensor_reduce(acc[:, 2 * c + 1:2 * c + 2], W4[:],
                             mybir.AxisListType.X, OP.add)

        nc.sync.dma_start(out_ap, acc[:])


_NC_CACHE = None


def build_nc():
    global _NC_CACHE
    if _NC_CACHE is not None:
        return _NC_CACHE
    nc = bacc.Bacc(
        "TRN2",
        target_bir_lowering=False,
        debug=False,
        enable_asserts=False,
        num_devices=NCORES,
    )
    CT = sum(CS)
    geo = nc.dram_tensor("geo", [P, GCH * CT], BF16, kind="ExternalInput")
    cls = nc.dram_tensor("cls", [P, CCH * CT], BF16, kind="ExternalInput")
    out = nc.dram_tensor("out", [P, 2 * NCHUNK], F32, kind="ExternalOutput")
    with tile.TileContext(nc) as tc:
        _body(tc, geo.ap(), cls.ap(), out.ap())
    nc.compile()
    _NC_CACHE = nc
    return nc


def make_in_maps(pred, labels):
    pred = np.asarray(pred, dtype=np.float32).reshape(B, 30, 49)
    labels = np.asarray(labels, dtype=np.float32).reshape(B, 30, 49)
    pg = [0, 5, 1, 6,          # x1 x2 y1 y2
          2, 7,                # w1 w2
          3, 8,                # h1 h2
          4, 9]                # c1 c2
    geo_parts, cls_parts = [], []
    r0 = 0
    for c, k in enumerate(KS):
        Cc = CS[c]
        rows = P * k
        # rows r0 .. r0+rows: partition p holds rows r0 + j*P + p, j<k
        def img(x, chans):
            n = len(chans)
            y = x[:, chans].reshape(NCORES, BLOC, n, 49)
            y = y[:, r0:r0 + rows].reshape(NCORES, k, P, n, 49)
            y = y.transpose(0, 2, 3, 1, 4)    # core, p, ch, k, 49
            return np.ascontiguousarray(y).reshape(NCORES, P, n, Cc)

        gp = img(pred, pg)
        gl = img(labels, [0, 1, 2, 3, 4])
        geo = np.empty((NCORES, P, GCH, Cc), dtype=np.float32)
        geo[:, :, 0:4] = gp[:, :, 0:4]                   # x1 x2 y1 y2
        geo[:, :, 4:6] = gp[:, :, 4:6]                   # w1 w2
        geo[:, :, 6] = gl[:, :, 2]                       # lw
        geo[:, :, 7] = gl[:, :, 2]                       # lw
        geo[:, :, 8:10] = gp[:, :, 6:8]                  # h1 h2
        geo[:, :, 10] = gl[:, :, 3]                      # lh
        geo[:, :, 11] = gl[:, :, 3]                      # lh
        geo[:, :, 12] = gl[:, :, 0]                      # lx
        geo[:, :, 13] = gl[:, :, 0]                      # lx
        geo[:, :, 14] = gl[:, :, 1]                      # ly
        geo[:, :, 15] = gl[:, :, 1]                      # ly
        geo[:, :, 16:18] = gp[:, :, 8:10]                # c1 c2
        geo[:, :, 18] = gl[:, :, 4]                      # obj
        geo_parts.append(geo.reshape(NCORES, P, GCH * Cc))
        clsb = np.concatenate(
            [img(pred, list(range(10, 30))),
             img(labels, list(range(10, 30)))], axis=2)
        cls_parts.append(clsb.reshape(NCORES, P, CCH * Cc))
        r0 += rows
    geo = np.concatenate(geo_parts, axis=2).astype(bfloat16)
    cls = np.concatenate(cls_parts, axis=2).astype(bfloat16)
    return [
        {"geo": np.ascontiguousarray(geo[i]),
         "cls": np.ascontiguousarray(cls[i])}
        for i in range(NCORES)
    ]


def run(pred, labels, trace=False, **kw):
    nc = build_nc()
    in_maps = make_in_maps(pred, labels)
    res = run_bass_kernel_spmd(
        nc, in_maps, core_ids=list(range(NCORES)), trace=trace, **kw)
    total = np.float64(0.0)
    for r in res.results:
        total += r["out"].astype(np.float64).sum()
    loss = np.float32(total / B)
    return loss, res


def kernel(pred, labels):
    loss, _ = run(pred, labels)
    return np.array(loss, dtype=np.float32)


# revision 30
# speedup vs baseline: 1.4697x; 1.0432x over previous
"""YOLOv1 loss kernel for Trainium2, data-parallel over 8 NeuronCores.

Full inputs: pred [16384,30,7,7] f32, labels [16384,30,7,7] f32 -> scalar f32.

Sharding: batch 16384 -> 8 cores x 2048 rows. Host packs each core's shard
into bf16 channel-major images (geo: 19 channels, cls: 40 channels) so every
on-chip op is one wide contiguous span (DVE 2x bf16 mode). Per core the
kernel streams NCHUNK chunks, computes the loss fully on chip into two f32
accumulators per chunk ([P] each), and the host sums partials / divides by B.

Math (equivalent to the reference up to rounding):
  - Grid offsets m,n cancel in the IOU; scaling coords by 7 cancels in
    inter/union. Interval overlap identity: min(hi)-max(lo) =
    3.5*(wp+wg) - |xp-xg|, so no lo/hi box corners are ever formed.
  - inter = relu(ovx)*relu(ovy), den = 49*(a+ag) - inter, iou = inter/den
    (den >= 49*ag > 0 always).  1/den via exp(-ln(den)) on ACT.
  - (sqrt(p)-sqrt(l))^2 = p + l - 2*sqrt(p*l): e5 = 5*(p+l) - 10*sqrt(p*l),
    with 10*sqrt(m) = exp(0.5*ln(100*m)).
  - Per cell: cell = sph + obj*(selU + S + cls - sph), sph = 0.5(c1^2+c2^2),
    selU = resp ? cc1+0.5objc1 : cc2+0.5objc2, S = 0.5objc1+0.5objc2.
    Sum(sph) via ACT accumulate; Sum(obj*(...)) via tensor_tensor_reduce.
"""

import numpy as np
from ml_dtypes import bfloat16

import concourse.bass as bass
import concourse.mybir as mybir
import concourse.tile as tile
from concourse import bacc
from concourse.bass_utils import run_bass_kernel_spmd

F32 = mybir.dt.float32
BF16 = mybir.dt.bfloat16
I32 = mybir.dt.int32
OP = mybir.AluOpType
AF = mybir.ActivationFunctionType

NCORES = 8
B = 16384
BLOC = B // NCORES        # 2048 rows per core
P = 128                   # SBUF partitions
KS = [5, 11]              # rows per partition per chunk (sum = BLOC/P)
NCHUNK = len(KS)
CS = [49 * k for k in KS]  # cells per partition per chunk
GCH = 19                  # geo channels
CCH = 40                  # cls channels (20 pred + 20 label)

SQ5 = float(np.float32(np.sqrt(5.0)))
ISQ2 = float(np.float32(np.sqrt(0.5)))

# how many cls channels the GPSIMD engine subtracts (rest on DVE)
GPS_SUB_CH = 4
# cls channels reduced by GPSIMD pool (window POOLW); rest tree-added on DVE
POOLW = 10


def _body(tc, geo_ap, cls_ap, out_ap):
    nc = tc.nc
    nv = nc.vector
    na = nc.scalar
    ng = nc.gpsimd

    import contextlib
    ctx = contextlib.ExitStack()
    with ctx:
        inp = ctx.enter_context(tc.tile_pool(name="inp", bufs=2))
        med = ctx.enter_context(tc.tile_pool(name="med", bufs=1))
        opool = ctx.enter_context(tc.tile_pool(name="opool", bufs=1))

        acc = opool.tile([P, 2 * NCHUNK], F32)
        CMAX = max(CS)

        for c in range(NCHUNK):
            C = CS[c]
            go = GCH * sum(CS[:c])
            lo_off = CCH * sum(CS[:c])
            GT = inp.tile([P, GCH * CMAX], BF16, tag="GT")
            LT = inp.tile([P, CCH * CMAX], BF16, tag="LT")
            nc.sync.dma_start(GT[:, 0:8 * C], geo_ap[:, go:go + 8 * C])
            nc.sync.dma_start(GT[:, 8 * C:GCH * C],
                              geo_ap[:, go + 8 * C:go + GCH * C])
            nc.sync.dma_start(LT[:, 0:CCH * C],
                              cls_ap[:, lo_off:lo_off + CCH * C])

            def g(a, b):
                # geo channel span [a, b)
                return GT[:, a * C:b * C]

            def t2(name, ch, pool=med, dt=BF16):
                t = pool.tile([P, ch * CMAX], dt, tag=name, name=name)
                return t[:, 0:ch * C]

            # ---- geometry ----
            D4 = t2("D4", 4)     # [x1-lx, x2-lx, y1-ly, y2-ly]
            nv.tensor_tensor(D4[:], g(0, 4), g(4, 8), OP.subtract)
            A4 = t2("A4", 4)     # |dxy|/3.5 -> ov -> rv (in place)
            na.activation(A4[:], D4[:], AF.Abs, scale=1.0 / 3.5)
            # w/h channel pair views: [[w1 w2],[h1 h2]] and [[lw lw],[lh lh]]
            gv = GT[:, 8 * C:16 * C].rearrange("p (b x) -> p b x", b=2)
            wh_p = gv[:, :, 0:2 * C]
            wh_l = gv[:, :, 2 * C:4 * C]

            def bv(t):
                return t.rearrange("p (b x) -> p b x", b=2)

            T4 = t2("T4", 4)     # [w1+lw, w2+lw, h1+lh, h2+lh]
            nv.tensor_tensor(bv(T4), wh_p, wh_l, OP.add)
            M = t2("M", 4)       # [w1*lw, w2*lw, h1*lh, h2*lh]
            nv.tensor_tensor(bv(M), wh_p, wh_l, OP.mult)
            AAG = t2("AAG", 4)   # [a1, a2, ag, ag]
            nv.tensor_tensor(AAG[:], g(8, 12), g(12, 16), OP.mult)

            # 2*sqrt(m) = exp(0.5*ln(4*m))   (in place over M)
            na.activation(M[:], M[:], AF.Ln, scale=4.0)
            na.activation(M[:], M[:], AF.Exp, scale=0.5)

            # ---- intersection / iou (coords scaled by 1/3.5) ----
            # ovs = min(t - |dxy|/3.5, 2*min(wp,wg)); rv = relu(ovs)
            MN = t2("MN", 4)
            nv.tensor_tensor(bv(MN), wh_p, wh_l, OP.min)
            nv.tensor_tensor(A4[:], T4[:], A4[:], OP.subtract)
            nv.scalar_tensor_tensor(A4[:], MN[:], 2.0, A4[:], OP.mult,
                                    OP.min)
            nv.tensor_scalar_max(A4[:], A4[:], 0.0)
            I2 = t2("I2", 2)     # inter/12.25 -> iou (in place)
            nv.tensor_tensor(I2[:], A4[:, 0:2 * C], A4[:, 2 * C:4 * C],
                             OP.mult)
            S2 = t2("S2", 2)     # a+ag -> den/12.25 -> ln -> 1/den (in place)
            nv.tensor_tensor(S2[:], AAG[:, 0:2 * C], AAG[:, 2 * C:4 * C],
                             OP.add)
            nv.scalar_tensor_tensor(S2[:], S2[:], 4.0, I2[:], OP.mult,
                                    OP.subtract)
            na.activation(S2[:], S2[:], AF.Ln)
            na.activation(S2[:], S2[:], AF.Exp, scale=-1.0)
            nv.tensor_tensor(I2[:], I2[:], S2[:], OP.mult)   # iou1, iou2

            RSP = t2("RSP", 1, dt=I32)
            nv.tensor_tensor(RSP[:], I2[:, 0:C], I2[:, C:2 * C], OP.is_ge)

            # objc'' = 0.1*(c - iou)^2
            DC = t2("DC", 2)
            nv.tensor_tensor(DC[:], g(16, 18), I2[:], OP.subtract)
            na.activation(DC[:], DC[:], AF.Square,
                          scale=float(np.sqrt(0.1)))

            # ---- coordinate loss (unscaled; the x5 is folded in at w2) ----
            DQ = t2("DQ", 8)     # [dxy^2 (4ch) | t - 2*sqrt(m) (4ch)]
            na.activation(DQ[:, 0:4 * C], D4[:], AF.Square)
            nv.tensor_tensor(DQ[:, 4 * C:8 * C], T4[:], M[:], OP.subtract)
            nv.tensor_tensor(DQ[:, 0:4 * C], DQ[:, 0:4 * C],
                             DQ[:, 4 * C:8 * C], OP.add)
            nv.tensor_tensor(DQ[:, 0:2 * C], DQ[:, 0:2 * C],
                             DQ[:, 2 * C:4 * C], OP.add)    # coor1, coor2
            nv.tensor_tensor(DQ[:, 0:2 * C], DQ[:, 0:2 * C], DC[:],
                             OP.add)                # coor_k + objc''_k

            SS1 = t2("SS1", 1)   # S'' = objc''1 + objc''2
            nv.tensor_tensor(SS1[:], DC[:, 0:C], DC[:, C:2 * C], OP.add)

            # sph = 0.5*(c1^2 + c2^2) per cell; noobj = 1 - obj
            H2 = t2("H2", 2)
            na.activation(H2[:], g(16, 18), AF.Square, scale=ISQ2)
            SPH = t2("SPH", 1)
            nv.tensor_tensor(SPH[:], H2[:, 0:C], H2[:, C:2 * C], OP.add)
            NOB = t2("NOB", 1)
            nv.tensor_scalar(NOB[:], g(18, 19), -1.0, 1.0, OP.mult, OP.add)

            # ---- cls: two interleaved 10-channel pipelines ----
            D20 = t2("D20", 20)
            oo = [0, 10 * C]

            def lvl(f):
                for o in oo:
                    f(o)

            lvl(lambda o: nv.tensor_tensor(
                D20[:, o:o + 10 * C], LT[:, o:o + 10 * C],
                LT[:, 20 * C + o:30 * C + o], OP.subtract))
            lvl(lambda o: na.activation(
                D20[:, o:o + 10 * C], D20[:, o:o + 10 * C], AF.Square))
            lvl(lambda o: nv.tensor_tensor(
                D20[:, o:o + 5 * C], D20[:, o:o + 5 * C],
                D20[:, o + 5 * C:o + 10 * C], OP.add))
            lvl(lambda o: nv.tensor_tensor(
                D20[:, o:o + 2 * C], D20[:, o:o + 2 * C],
                D20[:, o + 2 * C:o + 4 * C], OP.add))
            lvl(lambda o: nv.tensor_tensor(
                D20[:, o:o + C], D20[:, o:o + C],
                D20[:, o + C:o + 2 * C], OP.add))
            lvl(lambda o: nv.tensor_tensor(
                D20[:, o:o + C], D20[:, o:o + C],
                D20[:, o + 4 * C:o + 5 * C], OP.add))
            nv.tensor_tensor(D20[:, 0:C], D20[:, 0:C],
                             D20[:, 10 * C:11 * C], OP.add)  # cls total

            # ---- combine:
            # acc_a += sum(noobj * sph)
            # acc_b += sum(obj * (5*(sel(resp, cc1, cc2) + S'') + cls))
            V1 = t2("V1", 1)
            nv.tensor_copy(V1[:], DQ[:, C:2 * C])
            nv.copy_predicated(V1[:], RSP[:], DQ[:, 0:C])
            SC1 = t2("SC1", 1)
            nv.tensor_tensor(SC1[:], SPH[:], NOB[:], OP.mult)
            nv.tensor_reduce(acc[:, 2 * c:2 * c + 1], SC1[:],
                             mybir.AxisListType.X, OP.add)
            nv.tensor_tensor(V1[:], V1[:], SS1[:], OP.add)
            nv.scalar_tensor_tensor(V1[:], V1[:], 5.0, D20[:, 0:C],
                                    OP.mult, OP.add)
            W4 = t2("W4", 1)
            nv.tensor_tensor(W4[:], V1[:], g(18, 19), OP.mult)
            nv.tensor_reduce(acc[:, 2 * c + 1:2 * c + 2], W4[:],
                             mybir.AxisListType.X, OP.add)

        nc.sync.dma_start(out_ap, acc[:])


_NC_CACHE = None


def build_nc():
    global _NC_CACHE
    if _NC_CACHE is not None:
        return _NC_CACHE
    nc = bacc.Bacc(
        "TRN2",
        target_bir_lowering=False,
        debug=False,
        enable_asserts=False,
        num_devices=NCORES,
    )
    CT = sum(CS)
    geo = nc.dram_tensor("geo", [P, GCH * CT], BF16, kind="ExternalInput")
    cls = nc.dram_tensor("cls", [P, CCH * CT], BF16, kind="ExternalInput")
    out = nc.dram_tensor("out", [P, 2 * NCHUNK], F32, kind="ExternalOutput")
    with tile.TileContext(nc) as tc:
        _body(tc, geo.ap(), cls.ap(), out.ap())
    nc.compile()
    _NC_CACHE = nc
    return nc


def make_in_maps(pred, labels):
    pred = np.asarray(pred, dtype=np.float32).reshape(B, 30, 49)
    labels = np.asarray(labels, dtype=np.float32).reshape(B, 30, 49)
    pg = [0, 5, 1, 6,          # x1 x2 y1 y2
          2, 7,                # w1 w2
          3, 8,                # h1 h2
          4, 9]                # c1 c2
    geo_parts, cls_parts = [], []
    r0 = 0
    for c, k in enumerate(KS):
        Cc = CS[c]
        rows = P * k
        # rows r0 .. r0+rows: partition p holds rows r0 + j*P + p, j<k
        def img(x, chans):
            n = len(chans)
            y = x[:, chans].reshape(NCORES, BLOC, n, 49)
            y = y[:, r0:r0 + rows].reshape(NCORES, k, P, n, 49)
            y = y.transpose(0, 2, 3, 1, 4)    # core, p, ch, k, 49
            return np.ascontiguousarray(y).reshape(NCORES, P, n, Cc)

        gp = img(pred, pg)
        gl = img(labels, [0, 1, 2, 3, 4])
        geo = np.empty((NCORES, P, GCH, Cc), dtype=np.float32)
        geo[:, :, 0:4] = gp[:, :, 0:4]                   # x1 x2 y1 y2
        geo[:, :, 4] = gl[:, :, 0]                       # lx
        geo[:, :, 5] = gl[:, :, 0]                       # lx
        geo[:, :, 6] = gl[:, :, 1]                       # ly
        geo[:, :, 7] = gl[:, :, 1]                       # ly
        geo[:, :, 8:10] = gp[:, :, 4:6]                  # w1 w2
        geo[:, :, 10] = gl[:, :, 2]                      # lw
        geo[:, :, 11] = gl[:, :, 2]                      # lw
        geo[:, :, 12:14] = gp[:, :, 6:8]                 # h1 h2
        geo[:, :, 14] = gl[:, :, 3]                      # lh
        geo[:, :, 15] = gl[:, :, 3]                      # lh
        geo[:, :, 16:18] = gp[:, :, 8:10]                # c1 c2
        geo[:, :, 18] = gl[:, :, 4]                      # obj
        geo_parts.append(geo.reshape(NCORES, P, GCH * Cc))
        clsb = np.concatenate(
            [img(pred, list(range(10, 30))),
             img(labels, list(range(10, 30)))], axis=2)
        cls_parts.append(clsb.reshape(NCORES, P, CCH * Cc))
        r0 += rows
    geo = np.concatenate(geo_parts, axis=2).astype(bfloat16)
    cls = np.concatenate(cls_parts, axis=2).astype(bfloat16)
    return [
        {"geo": np.ascontiguousarray(geo[i]),
         "cls": np.ascontiguousarray(cls[i])}
        for i in range(NCORES)
    ]


def run(pred, labels, trace=False, **kw):
    nc = build_nc()
    in_maps = make_in_maps(pred, labels)
    res = run_bass_kernel_spmd(
        nc, in_maps, core_ids=list(range(NCORES)), trace=trace, **kw)
    total = np.float64(0.0)
    for r in res.results:
        total += r["out"].astype(np.float64).sum()
    loss = np.float32(total / B)
    return loss, res


def kernel(pred, labels):
    loss, _ = run(pred, labels)
    return np.array(loss, dtype=np.float32)
